# revision 4
# baseline (speedup 1.0000x reference)
"""CRF log-likelihood (sum over batch) on 8 Trainium2 NeuronCores.

Math (per batch element b):
    llh[b] = score(gold path) - logZ  (forward algorithm)
The forward recurrence is run on-device in the exp domain:
    u_0     = exp(start + em_0 - d)
    u_{t+1} = (u_t @ E) * exp(em_{t+1} - d),   E = exp(transitions)
    logZ    = log(sum_j u_{S-1}[j] * exp(end_j)) + S*d
where d is a constant per-step log-growth preconditioner (estimated on
host from 2 batch columns) that keeps u inside fp32/bf16 range, making
per-step renormalization (a partition-axis reduction) unnecessary.

Device mapping (per core, batch 64 = 2 groups of 32):
    partitions p = gi*64 + j  (gi in {0,1} batch half, j = tag)
    state u: [128, 32] bf16;  per step: one matmul with a block-diagonal
    stationary E+E [128,128] (q = u @ E for both groups at once), then one
    VectorE tensor_mul with the precomputed g = exp(em - d) slice.
    g is produced on-device by ScalarE Exp over DMA-streamed emissions.

The gold-path score (tiny gather-only term) is computed on host.
"""

import numpy as np
import ml_dtypes

import concourse.bacc as bacc
import concourse.mybir as mybir
import concourse.tile as tile
from concourse.bass_utils import run_bass_kernel_spmd

S, B, T = 1024, 512, 64
NCORES = 8
BPC = B // NCORES          # 64 batch elements per core
GB = BPC // 2              # 32 per partition-group
CHUNK = 64                 # time steps per DMA/exp chunk
NCHUNK = S // CHUNK

BF16 = ml_dtypes.bfloat16

_CACHE = {}


def build_nc():
    nc = bacc.Bacc("TRN2", target_bir_lowering=False, debug=False,
                   num_devices=NCORES)
    em = nc.dram_tensor("em", [128, S * GB], mybir.dt.float32,
                        kind="ExternalInput").ap()
    u0 = nc.dram_tensor("u0", [128, GB], mybir.dt.bfloat16,
                        kind="ExternalInput").ap()
    eblk = nc.dram_tensor("eblk", [128, 128], mybir.dt.bfloat16,
                          kind="ExternalInput").ap()
    negd = nc.dram_tensor("negd", [128, 1], mybir.dt.float32,
                          kind="ExternalInput").ap()
    uT = nc.dram_tensor("uT", [128, GB], mybir.dt.bfloat16,
                        kind="ExternalOutput").ap()

    with tile.TileContext(nc) as tc:
        with (
            tc.tile_pool(name="const", bufs=1) as constp,
            tc.tile_pool(name="g", bufs=NCHUNK) as gp,
            tc.tile_pool(name="stage", bufs=3) as stp,
            tc.tile_pool(name="u", bufs=4) as up,
            tc.tile_pool(name="q", bufs=4, space="PSUM") as qp,
        ):
            eb = constp.tile([128, 128], mybir.dt.bfloat16)
            nc.sync.dma_start(eb[:], eblk)
            nd = constp.tile([128, 1], mybir.dt.float32)
            nc.sync.dma_start(nd[:], negd)
            ut0 = constp.tile([128, GB], mybir.dt.bfloat16)
            nc.sync.dma_start(ut0[:], u0)

            gts = []
            for c in range(NCHUNK):
                stg = stp.tile([128, CHUNK * GB], mybir.dt.float32)
                nc.sync.dma_start(
                    stg[:], em[:, c * CHUNK * GB:(c + 1) * CHUNK * GB])
                gt = gp.tile([128, CHUNK * GB], mybir.dt.bfloat16)
                nc.scalar.activation(gt[:], stg[:],
                                     mybir.ActivationFunctionType.Exp,
                                     bias=nd[:], scale=1.0)
                gts.append(gt)

            u_prev = ut0
            for t in range(1, S):
                q = qp.tile([128, GB], mybir.dt.float32)
                nc.tensor.matmul(q[:], lhsT=eb[:], rhs=u_prev[:],
                                 start=True, stop=True)
                un = up.tile([128, GB], mybir.dt.bfloat16)
                g_ap = gts[t // CHUNK][:, (t % CHUNK) * GB:
                                       ((t % CHUNK) + 1) * GB]
                nc.vector.tensor_mul(un[:], q[:], g_ap)
                u_prev = un

            nc.sync.dma_start(uT, u_prev[:])
    nc.compile()
    return nc


def _get_nc():
    if "nc" not in _CACHE:
        _CACHE["nc"] = build_nc()
    return _CACHE["nc"]


def _estimate_d(em, st, tr):
    """Per-step log-growth of the forward recurrence, from 2 batch columns."""
    sub = em[:, :2, :].astype(np.float64)
    Ed = np.exp(tr.astype(np.float64))
    alpha = st.astype(np.float64)[None, :] + sub[0]
    for t in range(1, S):
        m = alpha.max(axis=1, keepdims=True)
        alpha = m + np.log(np.exp(alpha - m) @ Ed) + sub[t]
    return float(alpha.max(axis=1).mean() / S)


def _host_inputs(em, st, tr, d):
    """Per-core input maps for the device program."""
    E = np.exp(tr, dtype=np.float64)
    eblk = np.zeros((128, 128), np.float64)
    eblk[0:64, 0:64] = E
    eblk[64:128, 64:128] = E
    eblk = eblk.astype(BF16)
    negd = np.full((128, 1), -d, np.float32)

    in_maps = []
    for c in range(NCORES):
        x = em[:, BPC * c:BPC * (c + 1), :]                # (S, 64, T)
        xr = np.ascontiguousarray(
            x.reshape(S, 2, GB, T).transpose(1, 3, 0, 2)   # (gi, j, t, b')
        ).reshape(128, S * GB).astype(np.float32)
        u0 = np.exp(st[None, :].astype(np.float64)
                    + x[0].astype(np.float64) - d)          # (64b, T)
        u0 = np.ascontiguousarray(
            u0.reshape(2, GB, T).transpose(0, 2, 1)         # (gi, j, b')
        ).reshape(128, GB).astype(BF16)
        in_maps.append({"em": xr, "u0": u0, "eblk": eblk, "negd": negd})
    return in_maps


def _numerator(em, tags, mask_f, st, en, tr):
    emit = np.take_along_axis(
        em.astype(np.float64), tags[:, :, None].astype(np.int64), axis=2
    )[:, :, 0]
    score = st.astype(np.float64)[tags[0]] + emit[0]
    score = score + ((tr.astype(np.float64)[tags[:-1], tags[1:]] + emit[1:])
                     * mask_f[1:].astype(np.float64)).sum(0)
    seq_ends = mask_f.astype(np.int64).sum(0) - 1
    last_tags = tags[seq_ends, np.arange(tags.shape[1])]
    return score + en.astype(np.float64)[last_tags]


def _host_reference(em, tags, mask_f, st, en, tr):
    """Exact fp64 fallback (used only if mask is not all ones)."""
    Ed = np.exp(tr.astype(np.float64))
    alpha = st.astype(np.float64)[None, :] + em[0].astype(np.float64)
    for t in range(1, S):
        m = alpha.max(axis=1, keepdims=True)
        nxt = m + np.log(np.exp(alpha - m) @ Ed) + em[t].astype(np.float64)
        alpha = np.where(mask_f[t][:, None] > 0, nxt, alpha)
    m = alpha.max(axis=1)
    den = m + np.log(
        np.exp(alpha - m[:, None] + en.astype(np.float64)[None, :]).sum(1))
    num = _numerator(em, tags, mask_f, st, en, tr)
    return np.float32((num - den).sum())


def kernel(emissions, tags, mask, start_transitions, end_transitions,
           transitions):
    em = np.asarray(emissions, np.float32)
    tags = np.asarray(tags)
    mask = np.asarray(mask)
    st = np.asarray(start_transitions, np.float32)
    en = np.asarray(end_transitions, np.float32)
    tr = np.asarray(transitions, np.float32)
    mask_f = (mask != 0).astype(np.float32)

    if not bool((mask != 0).all()):
        return _host_reference(em, tags, mask_f, st, en, tr)

    d = _estimate_d(em, st, tr)
    in_maps = _host_inputs(em, st, tr, d)
    nc = _get_nc()
    results = run_bass_kernel_spmd(nc, in_maps,
                                   core_ids=list(range(NCORES))).results

    en64 = np.exp(en.astype(np.float64))
    den = np.empty(B, np.float64)
    for c in range(NCORES):
        uT = np.asarray(results[c]["uT"]).astype(np.float64)  # [128, GB]
        u = uT.reshape(2, T, GB)                              # (gi, j, b')
        r = np.einsum("gjb,j->gb", u, en64)                   # (2, GB)
        den[BPC * c:BPC * (c + 1)] = (np.log(r) + d * S).reshape(BPC)

    num = _numerator(em, tags, mask_f, st, en, tr)
    return np.float32((num - den).sum())


# revision 6
# speedup vs baseline: 1175.4357x; 1175.4357x over previous
"""CRF log-likelihood (sum over batch) on 8 Trainium2 NeuronCores.

Math (per batch element b):
    llh[b] = score(gold path) - logZ  (forward algorithm)
The forward recurrence is run on-device in the exp domain:
    u_0     = exp(start + em_0 - d)
    u_{t+1} = (u_t @ E) * exp(em_{t+1} - d),   E = exp(transitions)
    logZ    = log(sum_j u_{S-1}[j] * exp(end_j)) + S*d
where d is a constant per-step log-growth preconditioner (estimated on
host from 2 batch columns) that keeps u inside fp32/bf16 range, making
per-step renormalization (a partition-axis reduction) unnecessary.

Device mapping (per core, batch 64 = 2 groups of 32):
    partitions p = gi*64 + j  (gi in {0,1} batch half, j = tag)
    state u: [128, 32] bf16;  per step: one matmul with a block-diagonal
    stationary E+E [128,128] (q = u @ E for both groups at once), then one
    VectorE tensor_mul with the precomputed g = exp(em - d) slice.
    g is produced on-device by ScalarE Exp over DMA-streamed emissions.

The gold-path score (tiny gather-only term) is computed on host.
"""

import numpy as np
import ml_dtypes

import concourse.bacc as bacc
import concourse.mybir as mybir
import concourse.tile as tile
from concourse.bass_utils import run_bass_kernel_spmd

S, B, T = 1024, 512, 64
NCORES = 8
BPC = B // NCORES          # 64 batch elements per core
GB = BPC // 2              # 32 per partition-group
CHUNK = 64                 # time steps per DMA/exp chunk
NCHUNK = S // CHUNK

BF16 = ml_dtypes.bfloat16

_CACHE = {}


def build_nc(loop_reps=1):
    nc = bacc.Bacc("TRN2", target_bir_lowering=False, debug=False,
                   num_devices=NCORES)
    em = nc.dram_tensor("em", [128, S * GB], mybir.dt.float32,
                        kind="ExternalInput").ap()
    u0 = nc.dram_tensor("u0", [128, GB], mybir.dt.bfloat16,
                        kind="ExternalInput").ap()
    eblk = nc.dram_tensor("eblk", [128, 128], mybir.dt.bfloat16,
                          kind="ExternalInput").ap()
    negd = nc.dram_tensor("negd", [128, 1], mybir.dt.float32,
                          kind="ExternalInput").ap()
    uT = nc.dram_tensor("uT", [128, GB], mybir.dt.bfloat16,
                        kind="ExternalOutput").ap()

    with tile.TileContext(nc) as tc:
        with (
            tc.tile_pool(name="const", bufs=1) as constp,
            tc.tile_pool(name="g", bufs=NCHUNK) as gp,
            tc.tile_pool(name="stage", bufs=3) as stp,
            tc.tile_pool(name="u", bufs=4) as up,
            tc.tile_pool(name="q", bufs=4, space="PSUM") as qp,
        ):
            def body(_iv=None):
                eb = constp.tile([128, 128], mybir.dt.bfloat16)
                nc.sync.dma_start(eb[:], eblk)
                nd = constp.tile([128, 1], mybir.dt.float32)
                nc.sync.dma_start(nd[:], negd)
                ut0 = constp.tile([128, GB], mybir.dt.bfloat16)
                nc.sync.dma_start(ut0[:], u0)

                gts = []
                for c in range(NCHUNK):
                    stg = stp.tile([128, CHUNK * GB], mybir.dt.float32)
                    nc.sync.dma_start(
                        stg[:], em[:, c * CHUNK * GB:(c + 1) * CHUNK * GB])
                    gt = gp.tile([128, CHUNK * GB], mybir.dt.bfloat16)
                    nc.scalar.activation(gt[:], stg[:],
                                         mybir.ActivationFunctionType.Exp,
                                         bias=nd[:], scale=1.0)
                    gts.append(gt)

                u_prev = ut0
                for t in range(1, S):
                    q = qp.tile([128, GB], mybir.dt.float32)
                    nc.tensor.matmul(q[:], lhsT=eb[:], rhs=u_prev[:],
                                     start=True, stop=True)
                    un = up.tile([128, GB], mybir.dt.bfloat16)
                    g_ap = gts[t // CHUNK][:, (t % CHUNK) * GB:
                                           ((t % CHUNK) + 1) * GB]
                    nc.vector.tensor_mul(un[:], q[:], g_ap)
                    u_prev = un

                nc.sync.dma_start(uT, u_prev[:])

            if loop_reps == 1:
                body()
            else:
                with tc.For_i(0, loop_reps, 1):
                    body()
    nc.compile()
    return nc


def _get_nc():
    if "nc" not in _CACHE:
        _CACHE["nc"] = build_nc()
    return _CACHE["nc"]


def _estimate_d(em, st, tr):
    """Per-step log-growth of the forward recurrence, from 2 batch columns."""
    sub = em[:, :2, :].astype(np.float64)
    Ed = np.exp(tr.astype(np.float64))
    alpha = st.astype(np.float64)[None, :] + sub[0]
    for t in range(1, S):
        m = alpha.max(axis=1, keepdims=True)
        alpha = m + np.log(np.exp(alpha - m) @ Ed) + sub[t]
    return float(alpha.max(axis=1).mean() / S)


def _host_inputs(em, st, tr, d):
    """Per-core input maps for the device program."""
    E = np.exp(tr, dtype=np.float64)
    eblk = np.zeros((128, 128), np.float64)
    eblk[0:64, 0:64] = E
    eblk[64:128, 64:128] = E
    eblk = eblk.astype(BF16)
    negd = np.full((128, 1), -d, np.float32)

    in_maps = []
    for c in range(NCORES):
        x = em[:, BPC * c:BPC * (c + 1), :]                # (S, 64, T)
        xr = np.ascontiguousarray(
            x.reshape(S, 2, GB, T).transpose(1, 3, 0, 2)   # (gi, j, t, b')
        ).reshape(128, S * GB).astype(np.float32)
        u0 = np.exp(st[None, :].astype(np.float64)
                    + x[0].astype(np.float64) - d)          # (64b, T)
        u0 = np.ascontiguousarray(
            u0.reshape(2, GB, T).transpose(0, 2, 1)         # (gi, j, b')
        ).reshape(128, GB).astype(BF16)
        in_maps.append({"em": xr, "u0": u0, "eblk": eblk, "negd": negd})
    return in_maps


def _numerator(em, tags, mask_f, st, en, tr):
    emit = np.take_along_axis(
        em.astype(np.float64), tags[:, :, None].astype(np.int64), axis=2
    )[:, :, 0]
    score = st.astype(np.float64)[tags[0]] + emit[0]
    score = score + ((tr.astype(np.float64)[tags[:-1], tags[1:]] + emit[1:])
                     * mask_f[1:].astype(np.float64)).sum(0)
    seq_ends = mask_f.astype(np.int64).sum(0) - 1
    last_tags = tags[seq_ends, np.arange(tags.shape[1])]
    return score + en.astype(np.float64)[last_tags]


def _host_reference(em, tags, mask_f, st, en, tr):
    """Exact fp64 fallback (used only if mask is not all ones)."""
    Ed = np.exp(tr.astype(np.float64))
    alpha = st.astype(np.float64)[None, :] + em[0].astype(np.float64)
    for t in range(1, S):
        m = alpha.max(axis=1, keepdims=True)
        nxt = m + np.log(np.exp(alpha - m) @ Ed) + em[t].astype(np.float64)
        alpha = np.where(mask_f[t][:, None] > 0, nxt, alpha)
    m = alpha.max(axis=1)
    den = m + np.log(
        np.exp(alpha - m[:, None] + en.astype(np.float64)[None, :]).sum(1))
    num = _numerator(em, tags, mask_f, st, en, tr)
    return np.float32((num - den).sum())


def kernel(emissions, tags, mask, start_transitions, end_transitions,
           transitions):
    em = np.asarray(emissions, np.float32)
    tags = np.asarray(tags)
    mask = np.asarray(mask)
    st = np.asarray(start_transitions, np.float32)
    en = np.asarray(end_transitions, np.float32)
    tr = np.asarray(transitions, np.float32)
    mask_f = (mask != 0).astype(np.float32)

    if not bool((mask != 0).all()):
        return _host_reference(em, tags, mask_f, st, en, tr)

    d = _estimate_d(em, st, tr)
    in_maps = _host_inputs(em, st, tr, d)
    nc = _get_nc()
    results = run_bass_kernel_spmd(nc, in_maps,
                                   core_ids=list(range(NCORES))).results

    en64 = np.exp(en.astype(np.float64))
    den = np.empty(B, np.float64)
    for c in range(NCORES):
        uT = np.asarray(results[c]["uT"]).astype(np.float64)  # [128, GB]
        u = uT.reshape(2, T, GB)                              # (gi, j, b')
        r = np.einsum("gjb,j->gb", u, en64)                   # (2, GB)
        den[BPC * c:BPC * (c + 1)] = (np.log(r) + d * S).reshape(BPC)

    num = _numerator(em, tags, mask_f, st, en, tr)
    return np.float32((num - den).sum())
